# revision 12
# baseline (speedup 1.0000x reference)
"""Trainium2 Bass kernel for the minGRU encoder (nn_Encoder_65635690218112).

Strategy
--------
- Data-parallel over batch: 16 batches -> 8 cores x 2 batches.
- Everything is kept feature-major (h^T layout [D, T]): the input x arrives
  as [C_in, T] per batch and the output leaves as [C_out, T] per batch, so
  the whole pipeline is transpose-free.
- Matmuls run in float32r (full PE rate at N=512, ~1.4e-4 matmul rel err,
  ~3.8e-4 end-to-end vs the fp32 reference) accumulating into PSUM over 8
  k-blocks of 128.
- The minGRU recurrence h_t = a_t * h_{t-1} + b_t is computed with the
  hardware prefix-scan instruction (tensor_tensor_scan, op0=mult op1=add)
  on the vector engine, chained across 512-wide time chunks via `initial`.
- h lives in SBUF as 8 feature-blocks x 4 chunk tiles of [128, 512] f32r and
  is updated in place layer after layer (the scan of chunk c only runs after
  every matmul of chunk c has consumed the old h, which Tile enforces via
  WAR dependencies).
- The time axis is split in two groups of 2048 so that h (8.4MB) plus one
  layer of weights (8.4MB) fit in SBUF; per-layer carries [128,1] bridge the
  groups. Layer weights are streamed from DRAM into 16 single-buffered
  column tiles, which pipelines the next layer's loads behind the current
  layer's last reads.
"""

import numpy as np

import concourse.bass as bass
import concourse.mybir as mybir
import concourse.tile as tile

# ---------------------------------------------------------------------------
# Workaround: this walrus build accepts at most ONE sem wait per instruction
# ("Too many sync wait commands"). After Tile assigns waits, split any
# instruction carrying more by inserting same-engine NoOps ahead of it.
# ---------------------------------------------------------------------------
from concourse.vector_clock import ScopedClock

_MAX_WAITS = 1
_noop_ctr = [0]


def _split_waits_in_block(bb):
    new_list = []
    for inst in bb.instructions:
        si = getattr(inst, "sync_info", None)
        if si is not None and si.on_wait and len(si.on_wait) > _MAX_WAITS:
            waits = list(si.on_wait)
            keep = waits[-_MAX_WAITS:]
            extra = waits[:-_MAX_WAITS]
            for i in range(0, len(extra), _MAX_WAITS):
                _noop_ctr[0] += 1
                nop = mybir.InstNoOp(
                    name=f"I-waitsplit-{_noop_ctr[0]}",
                    engine=inst.engine,
                    bass_nofuse=True,
                    sync_info=mybir.SyncInfo(
                        on_wait=extra[i : i + _MAX_WAITS], on_update=[]
                    ),
                )
                new_list.append(nop)
            inst.sync_info = mybir.SyncInfo(on_wait=keep, on_update=si.on_update)
        new_list.append(inst)
    bb.instructions[:] = new_list


def _patched_drain_and_barrier(self, tick_clock, wait_clock):
    nc = self.nc
    drain_inst = nc.sync.drain()
    wait_clock.add_sem_waits(
        drain_inst.ins, ScopedClock({None: tick_clock.global_clock})
    )
    for bb in nc.main_func.blocks:
        _split_waits_in_block(bb)
    nc.all_engine_barrier()
    assert self.sems is not None
    popped = nc._tile_sem_poison_stack.pop()
    assert popped is self._sem_poison
    nc.clear_and_free_semaphores(list(self.sems.allocated().values()))
    nc.all_engine_barrier()


tile.TileContext._drain_and_barrier = _patched_drain_and_barrier

# ---------------------------------------------------------------------------

f32 = mybir.dt.float32
f32r = mybir.dt.float32r
AF = mybir.ActivationFunctionType
ALU = mybir.AluOpType

N_CORES = 8
B_FULL = 16
C_IN = 80
C_OUT = 194
D = 1024
NJ = D // 128  # 8 feature blocks of 128


def build_program(L=6, T=4096, G=2, S=512, BS=2, REP=1, mm_dt="f32r", fake_scan=False, static_w=False, layers_only=False):
    """Build the SPMD per-core Bass program. Returns nc.

    REP > 1 repeats the whole compute body (for differential timing); the
    program is idempotent so results are unchanged.
    """
    Tg = T // G
    NCH = Tg // S
    dmm = f32r if mm_dt == "f32r" else mybir.dt.bfloat16
    dio = f32 if mm_dt == "f32r" else mybir.dt.bfloat16
    nc = bass.Bass()

    x_d = nc.declare_dram_parameter("x", [BS, C_IN, T], dio, isOutput=False)
    wpre_d = nc.declare_dram_parameter("wpre", [C_IN, D], dio, isOutput=False)
    bpre_d = nc.declare_dram_parameter("bpre", [128, NJ], f32, isOutput=False)
    wl_d = nc.declare_dram_parameter("wl", [L, 2 * NJ, 128, D], dio, isOutput=False)
    bl_d = nc.declare_dram_parameter("bl", [L, 2, 128, NJ], f32, isOutput=False)
    wpost_d = nc.declare_dram_parameter(
        "wpost", [128, NJ * C_OUT], dio, isOutput=False
    )
    bpost_d = nc.declare_dram_parameter("bpost", [128, 2], f32, isOutput=False)
    out_d = nc.declare_dram_parameter("out", [BS, C_OUT, T], f32, isOutput=True)

    with tile.TileContext(nc) as tc:
        with (
            tc.tile_pool(name="const", bufs=1) as cpool,
            tc.tile_pool(name="h", bufs=1) as hpool,
            tc.tile_pool(name="w", bufs=1) as wpool,
            tc.tile_pool(name="bias", bufs=1) as bpool,
            tc.tile_pool(name="scr", bufs=1) as spool,
            tc.tile_pool(name="ps", bufs=1, space="PSUM") as pspool,
        ):
            # ---- constants loaded once ----
            wpre_sb = cpool.tile([C_IN, D], dmm, tag="wpre")
            nc.sync.dma_start(wpre_sb[:], wpre_d[:].bitcast(dmm))
            bpre_sb = cpool.tile([128, NJ], f32, tag="bpre")
            nc.sync.dma_start(bpre_sb[:], bpre_d[:])
            wpost_sb = cpool.tile([128, NJ * C_OUT], dmm, tag="wpost")
            nc.sync.dma_start(wpost_sb[:], wpost_d[:].bitcast(dmm))
            bpost_sb = cpool.tile([128, 2], f32, tag="bpost")
            nc.sync.dma_start(bpost_sb[:], bpost_d[:])
            carry_sb = cpool.tile([128, L * NJ], dmm, tag="carry")

            # persistent h tiles: [feature-block j][chunk c] of [128, S]
            h = [
                [hpool.tile([128, S], dmm, tag=f"h{j}_{c}", name=f"h{j}_{c}")
                 for c in range(NCH)]
                for j in range(NJ)
            ]

            for _rep in range(REP):
              for b in range(BS):
                for g in range(G):
                    t0 = g * Tg
                    # ---- input slab for this (batch, group) ----
                    x_sb = spool.tile([C_IN, Tg], dmm, tag="x", name="x_sb")
                    nc.sync.dma_start(
                        x_sb[:], x_d[b][:, t0 : t0 + Tg].bitcast(dmm)
                    )

                    # ---- pre-projection: h = x^T W_pre + b_pre (feature-major)
                    for c in range(NCH if not layers_only else 0):
                        for j in range(NJ):
                            ps = pspool.tile(
                                [128, S], f32,
                                tag=("psz" if j % 2 == 0 else "psc"), bufs=4,
                                name="ps_pre",
                            )
                            nc.tensor.matmul(
                                ps[:],
                                wpre_sb[:, j * 128 : (j + 1) * 128],
                                x_sb[:, c * S : (c + 1) * S],
                                start=True,
                                stop=True,
                            )
                            if j % 2 == 0:
                                nc.scalar.activation(
                                    h[j][c][:], ps[:], AF.Identity,
                                    bias=bpre_sb[:, j : j + 1], scale=1.0,
                                )
                            else:
                                nc.vector.tensor_scalar(
                                    h[j][c][:], ps[:],
                                    bpre_sb[:, j : j + 1], None, op0=ALU.add,
                                )

                    # ---- the L minGRU layers ----
                    for i in range(L):
                        if static_w and (b > 0 or g > 0 or i > 0 or _rep > 0):
                            pass  # reuse layer-0 weights (debug variant)
                        else:
                          wcols = []
                          for f in range(2 * NJ):
                            wt = wpool.tile(
                                [128, D], dmm, tag=f"w{f}", name=f"w{f}"
                            )
                            nc.sync.dma_start(wt[:], wl_d[i, f].bitcast(dmm))
                            wcols.append(wt)
                        bz = bpool.tile([128, NJ], f32, tag="bz", bufs=2, name="bz")
                        nc.sync.dma_start(bz[:], bl_d[i, 0])
                        bc = bpool.tile([128, NJ], f32, tag="bc", bufs=2, name="bc")
                        nc.sync.dma_start(bc[:], bl_d[i, 1])
                        nbz = bpool.tile([128, NJ], f32, tag="nbz", bufs=2, name="nbz")
                        nc.scalar.mul(nbz[:], bz[:], -1.0)

                        for c in range(NCH):
                            a_ts, b_ts = [], []
                            for j in range(NJ):
                                psz = pspool.tile(
                                    [128, S], f32, tag="psz", bufs=4, name="psz"
                                )
                                psc = pspool.tile(
                                    [128, S], f32, tag="psc", bufs=4, name="psc"
                                )
                                for kb in range(NJ):
                                    nc.tensor.matmul(
                                        psz[:],
                                        wcols[j][:, kb * 128 : (kb + 1) * 128],
                                        h[kb][c][:],
                                        start=(kb == 0),
                                        stop=(kb == NJ - 1),
                                    )
                                for kb in range(NJ):
                                    nc.tensor.matmul(
                                        psc[:],
                                        wcols[NJ + j][:, kb * 128 : (kb + 1) * 128],
                                        h[kb][c][:],
                                        start=(kb == 0),
                                        stop=(kb == NJ - 1),
                                    )
                                z_t = spool.tile(
                                    [128, S], f32, tag="z", bufs=4, name="z_t"
                                )
                                a_t = spool.tile(
                                    [128, S], f32, tag="a", bufs=9, name="a_t"
                                )
                                b_t = spool.tile(
                                    [128, S], f32, tag="bb", bufs=9, name="b_t"
                                )
                                # z = sigmoid(zh_z + bz)
                                nc.scalar.activation(
                                    z_t[:], psz[:], AF.Sigmoid,
                                    bias=bz[:, j : j + 1], scale=1.0,
                                )
                                # a = 1 - z = sigmoid(-(zh_z + bz))
                                nc.scalar.activation(
                                    a_t[:], psz[:], AF.Sigmoid,
                                    bias=nbz[:, j : j + 1], scale=-1.0,
                                )
                                # b = z * (zh_c + bc)
                                nc.vector.scalar_tensor_tensor(
                                    b_t[:], psc[:], bc[:, j : j + 1], z_t[:],
                                    op0=ALU.add, op1=ALU.mult,
                                )
                                a_ts.append(a_t)
                                b_ts.append(b_t)
                            # scans run after ALL matmuls of this chunk
                            for j in range(NJ):
                                if g == 0 and c == 0:
                                    init = 0.0
                                elif c == 0:
                                    init = carry_sb[:, i * NJ + j : i * NJ + j + 1]
                                else:
                                    init = h[j][c - 1][:, S - 1 : S]
                                if fake_scan:
                                    nc.vector.scalar_tensor_tensor(
                                        h[j][c][:], a_ts[j][:], 1.0, b_ts[j][:],
                                        op0=ALU.mult, op1=ALU.add,
                                    )
                                else:
                                    nc.vector.tensor_tensor_scan(
                                        h[j][c][:], a_ts[j][:], b_ts[j][:], init,
                                        op0=ALU.mult, op1=ALU.add,
                                    )
                        if g == 0:
                            for j in range(NJ):
                                nc.vector.tensor_copy(
                                    carry_sb[:, i * NJ + j : i * NJ + j + 1],
                                    h[j][NCH - 1][:, S - 1 : S],
                                )

                    # ---- post-projection: out = h^T W_post + b_post ----
                    if layers_only:
                        for c in range(NCH):
                            nc.sync.dma_start(
                                out_d[b][0:128, t0 + c * S : t0 + (c + 1) * S].bitcast(dmm),
                                h[0][c][:],
                            )
                        continue
                    for c in range(NCH):
                        for p, (p0, pw) in enumerate(((0, 128), (128, C_OUT - 128))):
                            ps_o = pspool.tile(
                                [128, S], f32,
                                tag=("psz" if p == 0 else "psc"), bufs=4,
                                name="ps_o",
                            )
                            for kb in range(NJ):
                                nc.tensor.matmul(
                                    ps_o[:pw, :],
                                    wpost_sb[
                                        :, kb * C_OUT + p0 : kb * C_OUT + p0 + pw
                                    ],
                                    h[kb][c][:],
                                    start=(kb == 0),
                                    stop=(kb == NJ - 1),
                                )
                            o_t = spool.tile([128, S], f32, tag="o", bufs=4, name="o_t")
                            if p == 0:
                                nc.scalar.activation(
                                    o_t[:pw, :], ps_o[:pw, :], AF.Identity,
                                    bias=bpost_sb[:pw, p : p + 1], scale=1.0,
                                )
                            else:
                                nc.vector.tensor_scalar(
                                    o_t[:pw, :], ps_o[:pw, :],
                                    bpost_sb[:pw, p : p + 1], None, op0=ALU.add,
                                )
                            nc.sync.dma_start(
                                out_d[b][p0 : p0 + pw, t0 + c * S : t0 + (c + 1) * S],
                                o_t[:pw, :],
                            )
    return nc


# ---------------------------------------------------------------------------
# fp8 DoubleRow variant
#
# Layer matmuls run in fp8e4m3 with the DoubleRow perf mode (2 fp8 k-elements
# per PE row -> 0.5 cycles/row, 2x the f32r rate).  Numerics (validated
# offline against the fp32 reference, max-rel ~3.5e-3 vs the 2e-2 gate):
#   - gate path zh_z:   plain fp8 W x fp8 h_hi            (4 DoubleRows)
#   - candidate zh_c:   split3 = Whi*hhi + Wlo*hhi + Whi*hlo (12 DoubleRows)
# where Xhi = fp8(X*scale), Xlo = fp8(X*scale - Xhi) (residual at same scale).
# Per-layer power-of-2 scales keep fp8 operands in range ~[-160, 160].
#
# All elementwise work is spread so no engine exceeds the PE's 4096
# cycles/(j,c)-tile:  Act: z=Sigmoid, c=Identity(+bias,dequant-scale);
# Pool: a=1-z, hhi=tensor_copy(h');  DVE: b=z*c, scan, hlo=h'-hhi.
# h' tiles are bf16 and live in SCALED units (h * 2^sh_next) so the fp8
# casts are pure copies; biases/scales are folded into Act's scale port.
# ---------------------------------------------------------------------------

f8 = mybir.dt.float8e4
bf16 = mybir.dt.bfloat16
u8 = mybir.dt.uint8
u16 = mybir.dt.uint16
NQ = NJ // 2  # 4 k-pairs of 256
SW = 10  # weight scale 2^10  (max|w|*2^10 ~ 107 < 240)
SH = [7, 8, 9, 10, 11, 12]  # per-layer input-h scales (max|h|*2^sh ~ 96-138)
DR = mybir.MatmulPerfMode.DoubleRow


def build_program_fp8(L=6, T=4096, G=2, S=512, BS=2, REP=1, pool_a=True):
    Tg = T // G
    NCH = Tg // S
    f32r = mybir.dt.float32r
    nc = bass.Bass()

    x_d = nc.declare_dram_parameter("x", [BS, C_IN, T], f32, isOutput=False)
    wpre_d = nc.declare_dram_parameter("wpre", [C_IN + 1, D], f32, isOutput=False)
    wz_d = nc.declare_dram_parameter("wz8", [L, 128, NQ * 2 * D], u8, isOutput=False)
    wch_d = nc.declare_dram_parameter("wch8", [L, 128, NQ * 2 * D], u8, isOutput=False)
    wcl_d = nc.declare_dram_parameter("wcl8", [L, 128, NQ * 2 * D], u8, isOutput=False)
    bz_d = nc.declare_dram_parameter("bz", [L, 128, NJ], f32, isOutput=False)
    bc_d = nc.declare_dram_parameter("bcs", [L, 128, NJ], f32, isOutput=False)
    wpost_d = nc.declare_dram_parameter("wpost", [128, NJ * C_OUT], u16, isOutput=False)
    bpost_d = nc.declare_dram_parameter("bpost", [1, C_OUT], u16, isOutput=False)
    out_d = nc.declare_dram_parameter("out", [BS, C_OUT, T], f32, isOutput=True)

    with tile.TileContext(nc) as tc:
        with (
            tc.tile_pool(name="const", bufs=1) as cpool,
            tc.tile_pool(name="h", bufs=1) as hpool,
            tc.tile_pool(name="h8", bufs=1) as h8pool,
            tc.tile_pool(name="w", bufs=2) as wpool,
            tc.tile_pool(name="bias", bufs=2) as bpool,
            tc.tile_pool(name="scr", bufs=1) as spool,
            tc.tile_pool(name="ps", bufs=1, space="PSUM") as pspool,
        ):
            # ---- constants ----
            wpre_sb = cpool.tile([C_IN + 1, D], f32r, tag="wpre")
            nc.sync.dma_start(wpre_sb[:], wpre_d[:].bitcast(f32r))
            wpost_sb = cpool.tile([128, NJ * C_OUT], bf16, tag="wpost")
            nc.sync.dma_start(wpost_sb[:], wpost_d[:].bitcast(bf16))
            bpost_sb = cpool.tile([1, C_OUT], bf16, tag="bpost")
            nc.sync.dma_start(bpost_sb[:], bpost_d[:].bitcast(bf16))
            ones_sb = cpool.tile([1, S], bf16, tag="ones")
            nc.vector.memset(ones_sb[:], 1.0)
            carry_sb = cpool.tile([128, L * NJ], bf16, tag="carry")

            # persistent h' (bf16, scaled units) and fp8 hi/lo tiles.
            # h8 tiles are double-buffered by layer parity so the casts for
            # layer i+1 never WAR against layer i's matmul reads.
            h = [
                [hpool.tile([128, S], bf16, tag=f"h{j}_{c}", name=f"h{j}_{c}")
                 for c in range(NCH)]
                for j in range(NJ)
            ]
            hhi = [
                [h8pool.tile([128, NJ, S], f8, tag=f"hhi{p}_{c}",
                             name=f"hhi{p}_{c}") for c in range(NCH)]
                for p in range(2)
            ]
            hlo = [
                [h8pool.tile([128, NJ, S], f8, tag=f"hlo{p}_{c}",
                             name=f"hlo{p}_{c}") for c in range(NCH)]
                for p in range(2)
            ]

            slab = 0
            for _rep in range(REP):
              for b in range(BS):
                for g in range(G):
                    t0 = g * Tg
                    x_sb = spool.tile([C_IN + 1, Tg], f32r, tag="x", bufs=2,
                                      name="x_sb")
                    if slab < 2:  # ones row persists per rotating buffer
                        # (memset must start at partition 0; row C_IN alone
                        # is not a legal base partition; f32r memset is not
                        # a valid ISA value type, so memset the f32 view)
                        nc.vector.memset(x_sb[:, :].bitcast(f32), 1.0)
                    nc.sync.dma_start(
                        x_sb[:C_IN, :], x_d[b][:, t0 : t0 + Tg].bitcast(f32r)
                    )
                    slab += 1

                    # ---- pre-projection (f32r): h0' = (x^T Wpre + bpre)*2^sh0
                    for c in range(NCH):
                        for j in range(NJ):
                            ps = pspool.tile(
                                [128, S], f32,
                                tag=(f"psz{c}" if j % 2 == 0 else f"psc{c}"),
                                bufs=1, name="ps_pre",
                            )
                            nc.tensor.matmul(
                                ps[:],
                                wpre_sb[:, j * 128 : (j + 1) * 128],
                                x_sb[:, c * S : (c + 1) * S],
                                start=True,
                                stop=True,
                            )
                            nc.scalar.activation(
                                h[j][c][:], ps[:], AF.Copy,
                                scale=float(2.0 ** SH[0]),
                            )
                            nc.gpsimd.tensor_copy(
                                hhi[0][c][:, j, :], h[j][c][:]
                            )
                            nc.vector.tensor_tensor(
                                hlo[0][c][:, j, :], h[j][c][:],
                                hhi[0][c][:, j, :], op=ALU.subtract,
                            )

                    # ---- the L minGRU layers ----
                    for i in range(L):
                        wz_sb = wpool.tile([128, NQ, 2, D], f8, tag="wz", name="wz")
                        nc.sync.dma_start(wz_sb[:], wz_d[i].bitcast(f8))
                        wch_sb = wpool.tile([128, NQ, 2, D], f8, tag="wch", name="wch")
                        nc.sync.dma_start(wch_sb[:], wch_d[i].bitcast(f8))
                        wcl_sb = wpool.tile([128, NQ, 2, D], f8, tag="wcl", name="wcl")
                        nc.sync.dma_start(wcl_sb[:], wcl_d[i].bitcast(f8))
                        bz_sb = bpool.tile([128, NJ], f32, tag="bz", name="bz")
                        nc.sync.dma_start(bz_sb[:], bz_d[i])
                        bc_sb = bpool.tile([128, NJ], f32, tag="bc", name="bc")
                        nc.sync.dma_start(bc_sb[:], bc_d[i])

                        sc_in = float(2.0 ** (-(SW + SH[i])))
                        sh_next = SH[i + 1] if i + 1 < L else 0
                        sc_c = float(2.0 ** (sh_next - SW - SH[i]))
                        last = i == L - 1
                        rd, wr = i % 2, (i + 1) % 2

                        for j in range(NJ):
                            jsl = slice(j * 128, (j + 1) * 128)
                            psz_t = [
                                pspool.tile([128, S], f32, tag=f"psz{c}",
                                            bufs=1, name=f"psz{c}")
                                for c in range(NCH)
                            ]
                            psc_t = [
                                pspool.tile([128, S], f32, tag=f"psc{c}",
                                            bufs=1, name=f"psc{c}")
                                for c in range(NCH)
                            ]

                            def pair(t, c, q):
                                return t[rd][c][:, 2 * q : 2 * q + 2, :]

                            # gate: Wz x hhi -- each weight slice loaded once,
                            # reused across the NCH chunks (ldweights=False)
                            for q in range(NQ):
                                for c in range(NCH):
                                    mm = nc.tensor.matmul(
                                        psz_t[c][:], wz_sb[:, q, :, jsl],
                                        pair(hhi, c, q),
                                        start=(q == 0), stop=(q == NQ - 1),
                                        perf_mode=DR,
                                    )
                                    if c > 0:
                                        mm.ins.ldweights = False
                            # candidate phase 1: Wc_hi x (hhi then hlo) --
                            # same weight load covers both products
                            for q in range(NQ):
                                for k, hsrc in enumerate((hhi, hlo)):
                                    for c in range(NCH):
                                        mm = nc.tensor.matmul(
                                            psc_t[c][:], wch_sb[:, q, :, jsl],
                                            pair(hsrc, c, q),
                                            start=(q == 0 and k == 0),
                                            stop=False,
                                            perf_mode=DR,
                                        )
                                        if k > 0 or c > 0:
                                            mm.ins.ldweights = False
                            # candidate phase 2: Wc_lo x hhi
                            for q in range(NQ):
                                for c in range(NCH):
                                    mm = nc.tensor.matmul(
                                        psc_t[c][:], wcl_sb[:, q, :, jsl],
                                        pair(hhi, c, q),
                                        start=False, stop=(q == NQ - 1),
                                        perf_mode=DR,
                                    )
                                    if c > 0:
                                        mm.ins.ldweights = False

                            for c in range(NCH):
                                z_t = spool.tile(
                                    [128, S], bf16, tag="z", bufs=4, name="z_t"
                                )
                                c_t = spool.tile(
                                    [128, S], bf16, tag="cc", bufs=4, name="c_t"
                                )
                                a_t = spool.tile(
                                    [128, S], bf16, tag="a", bufs=5, name="a_t"
                                )
                                b_t = spool.tile(
                                    [128, S], bf16, tag="bb", bufs=5, name="b_t"
                                )
                                nc.scalar.activation(
                                    z_t[:], psz_t[c][:], AF.Sigmoid,
                                    bias=bz_sb[:, j : j + 1], scale=sc_in,
                                )
                                nc.scalar.activation(
                                    c_t[:], psc_t[c][:], AF.Identity,
                                    bias=bc_sb[:, j : j + 1], scale=sc_c,
                                )
                                if pool_a:
                                    nc.gpsimd.tensor_scalar(
                                        a_t[:], z_t[:], -1.0, 1.0,
                                        op0=ALU.mult, op1=ALU.add,
                                    )
                                else:
                                    nc.vector.tensor_scalar(
                                        a_t[:], z_t[:], -1.0, 1.0,
                                        op0=ALU.mult, op1=ALU.add,
                                    )
                                nc.vector.tensor_tensor(
                                    b_t[:], z_t[:], c_t[:], op=ALU.mult
                                )
                                if g == 0 and c == 0:
                                    init = 0.0
                                elif c == 0:
                                    init = carry_sb[:, i * NJ + j : i * NJ + j + 1]
                                else:
                                    init = h[j][c - 1][:, S - 1 : S]
                                nc.vector.tensor_tensor_scan(
                                    h[j][c][:], a_t[:], b_t[:], init,
                                    op0=ALU.mult, op1=ALU.add,
                                )
                                if not last:
                                    nc.gpsimd.tensor_copy(
                                        hhi[wr][c][:, j, :], h[j][c][:]
                                    )
                                    nc.vector.tensor_tensor(
                                        hlo[wr][c][:, j, :], h[j][c][:],
                                        hhi[wr][c][:, j, :], op=ALU.subtract,
                                    )
                            if g == 0:
                                nc.vector.tensor_copy(
                                    carry_sb[:, i * NJ + j : i * NJ + j + 1],
                                    h[j][NCH - 1][:, S - 1 : S],
                                )

                    # ---- post-projection (bf16): out = h^T Wpost + bpost ----
                    for c in range(NCH):
                        for p, (p0, pw) in enumerate(((0, 128), (128, C_OUT - 128))):
                            ps_o = pspool.tile(
                                [128, S], f32,
                                tag=(f"psz{c}" if p == 0 else f"psc{c}"),
                                bufs=1, name="ps_o",
                            )
                            for kb in range(NJ):
                                nc.tensor.matmul(
                                    ps_o[:pw, :],
                                    wpost_sb[
                                        :, kb * C_OUT + p0 : kb * C_OUT + p0 + pw
                                    ],
                                    h[kb][c][:],
                                    start=(kb == 0),
                                    stop=False,
                                )
                            nc.tensor.matmul(
                                ps_o[:pw, :],
                                bpost_sb[:, p0 : p0 + pw],
                                ones_sb[:],
                                start=False,
                                stop=True,
                            )
                            o_t = spool.tile([128, S], f32, tag="o", bufs=4,
                                             name="o_t")
                            nc.scalar.activation(
                                o_t[:pw, :], ps_o[:pw, :], AF.Copy, scale=1.0
                            )
                            nc.sync.dma_start(
                                out_d[b][p0 : p0 + pw, t0 + c * S : t0 + (c + 1) * S],
                                o_t[:pw, :],
                            )
    return nc


def pack_inputs_fp8(x, w_pre, b_pre, w_layers, b_layers, w_post, b_post, L=6):
    import ml_dtypes

    fp8np = ml_dtypes.float8_e4m3
    bf16np = ml_dtypes.bfloat16
    f32 = np.float32

    x = np.ascontiguousarray(np.asarray(x, dtype=f32))
    wpre81 = np.empty((C_IN + 1, D), dtype=f32)
    wpre81[:C_IN] = np.asarray(w_pre, dtype=f32)
    wpre81[C_IN] = np.asarray(b_pre, dtype=f32)

    wl = np.asarray(w_layers, dtype=f32)
    sw = 2.0 ** SW

    def pack_w(wm):  # wm: [L, D, D] -> [L, 128, NQ*2*D] uint8 view of fp8
        # fp8 values already computed; layout [p, q, pair, m]
        out = np.empty((L, 128, NQ * 2 * D), dtype=np.uint8)
        for i in range(L):
            wi = wm[i]  # [D, D] fp8
            r = wi.reshape(NQ, 2, 128, D).transpose(2, 0, 1, 3).reshape(128, -1)
            out[i] = np.ascontiguousarray(r).view(np.uint8)
        return out

    wz_f = wl[:, :, :D] * sw
    wc_f = wl[:, :, D:] * sw
    wz8 = wz_f.astype(fp8np)
    wch8 = wc_f.astype(fp8np)
    wcl8 = (wc_f - wch8.astype(f32)).astype(fp8np)

    wz8 = pack_w(wz8)
    wch8p = pack_w(wch8)
    wcl8p = pack_w(wcl8)

    bl = np.asarray(b_layers, dtype=f32)  # [L, 2D]
    bz = np.ascontiguousarray(
        bl[:, :D].reshape(L, NJ, 128).transpose(0, 2, 1)
    )  # [L, 128, NJ]
    bcs = bl[:, D:].reshape(L, NJ, 128).transpose(0, 2, 1).copy()
    for i in range(L):
        sh_next = SH[i + 1] if i + 1 < L else 0
        bcs[i] *= 2.0 ** sh_next
    bcs = np.ascontiguousarray(bcs)

    wpost = (
        np.asarray(w_post, dtype=f32)
        .reshape(NJ, 128, C_OUT)
        .transpose(1, 0, 2)
        .reshape(128, NJ * C_OUT)
    )
    wpost = np.ascontiguousarray(wpost.astype(bf16np)).view(np.uint16)
    bpost = np.ascontiguousarray(
        np.asarray(b_post, dtype=f32).reshape(1, C_OUT).astype(bf16np)
    ).view(np.uint16)
    return {
        "x": x, "wpre": wpre81, "wz8": wz8, "wch8": wch8p, "wcl8": wcl8p,
        "bz": bz, "bcs": bcs, "wpost": wpost, "bpost": bpost,
    }


def pack_inputs(x, w_pre, b_pre, w_layers, b_layers, w_post, b_post, L=6, mm_dt="f32r"):
    """Host-side packing into DMA-friendly layouts (all contiguous 2D)."""
    if mm_dt == "f32r":
        io_np = np.float32
    else:
        import ml_dtypes
        io_np = ml_dtypes.bfloat16
    x = np.ascontiguousarray(np.asarray(x, dtype=np.float32).astype(io_np))
    w_pre = np.ascontiguousarray(np.asarray(w_pre, dtype=np.float32).astype(io_np))
    # wl[i, f, kp, kb*128+m] = w_layers[i, kb*128+kp, f*128+m]
    wl = (
        np.asarray(w_layers, dtype=np.float32)
        .reshape(L, NJ, 128, 2 * NJ, 128)
        .transpose(0, 3, 2, 1, 4)
        .reshape(L, 2 * NJ, 128, D)
    )
    wl = np.ascontiguousarray(wl.astype(io_np))
    bl = np.asarray(b_layers, dtype=np.float32).reshape(L, 2, NJ, 128)
    bl = np.ascontiguousarray(bl.transpose(0, 1, 3, 2))  # [L, 2, 128, NJ]
    bpre = np.ascontiguousarray(
        np.asarray(b_pre, dtype=np.float32).reshape(NJ, 128).T
    )  # [128, NJ]
    # wpost[kp, kb*C_OUT + c] = w_post[kb*128+kp, c]
    wpost = (
        np.asarray(w_post, dtype=np.float32)
        .reshape(NJ, 128, C_OUT)
        .transpose(1, 0, 2)
        .reshape(128, NJ * C_OUT)
    )
    wpost = np.ascontiguousarray(wpost.astype(io_np))
    bpost = np.zeros((128, 2), dtype=np.float32)
    bpost[:, 0] = np.asarray(b_post[:128], dtype=np.float32)
    bpost[: C_OUT - 128, 1] = np.asarray(b_post[128:], dtype=np.float32)
    return x, w_pre, bpre, wl, bl, wpost, bpost


_program_cache = {}


def _get_program(key):
    if key not in _program_cache:
        L, T, G, S, BS, REP, mm_dt = key[:7]
        fake = key[7] if len(key) > 7 else False
        statw = key[8] if len(key) > 8 else False
        lonly = key[9] if len(key) > 9 else False
        _program_cache[key] = build_program(L=L, T=T, G=G, S=S, BS=BS, REP=REP, mm_dt=mm_dt, fake_scan=fake, static_w=statw, layers_only=lonly)
    return _program_cache[key]


def run(inputs, L=6, T=4096, G=2, S=512, REP=1, mm_dt="fp8", trace=False,
        pool_a=True, **kw):
    """Run the SPMD kernel on the full inputs; returns (out, bass_results)."""
    from concourse.bass_utils import run_bass_kernel_spmd

    if mm_dt == "fp8":
        packed = pack_inputs_fp8(
            inputs["x"], inputs["w_pre"], inputs["b_pre"], inputs["w_layers"],
            inputs["b_layers"], inputs["w_post"], inputs["b_post"], L=L,
        )
        x = packed.pop("x")
        B = x.shape[0]
        BS = B // N_CORES
        key = (L, T, G, S, BS, REP, "fp8", pool_a)
        if key not in _program_cache:
            _program_cache[key] = build_program_fp8(
                L=L, T=T, G=G, S=S, BS=BS, REP=REP, pool_a=pool_a
            )
        nc = _program_cache[key]
        in_maps = [
            {"x": np.ascontiguousarray(x[c * BS : (c + 1) * BS]), **packed}
            for c in range(N_CORES)
        ]
    else:
        x, w_pre, bpre, wl, bl, wpost, bpost = pack_inputs(
            inputs["x"], inputs["w_pre"], inputs["b_pre"], inputs["w_layers"],
            inputs["b_layers"], inputs["w_post"], inputs["b_post"], L=L,
            mm_dt=mm_dt,
        )
        B = x.shape[0]
        BS = B // N_CORES
        nc = _get_program((L, T, G, S, BS, REP, mm_dt))
        shared = {"wpre": w_pre, "bpre": bpre, "wl": wl, "bl": bl,
                  "wpost": wpost, "bpost": bpost}
        in_maps = [
            {"x": np.ascontiguousarray(x[c * BS : (c + 1) * BS]), **shared}
            for c in range(N_CORES)
        ]
    res = run_bass_kernel_spmd(
        nc, in_maps, list(range(N_CORES)), trace=trace, **kw
    )
    out = np.concatenate([res.results[c]["out"] for c in range(N_CORES)], axis=0)
    return out, res


def kernel(**inputs):
    out, _ = run(inputs)
    return out



# revision 24
# speedup vs baseline: 1.3956x; 1.3956x over previous
"""Trainium2 Bass kernel for the minGRU encoder (nn_Encoder_65635690218112).

Strategy
--------
- Data-parallel over batch: 16 batches -> 8 cores x 2 batches.
- Everything is kept feature-major (h^T layout [D, T]): the input x arrives
  as [C_in, T] per batch and the output leaves as [C_out, T] per batch, so
  the whole pipeline is transpose-free.
- Matmuls run in float32r (full PE rate at N=512, ~1.4e-4 matmul rel err,
  ~3.8e-4 end-to-end vs the fp32 reference) accumulating into PSUM over 8
  k-blocks of 128.
- The minGRU recurrence h_t = a_t * h_{t-1} + b_t is computed with the
  hardware prefix-scan instruction (tensor_tensor_scan, op0=mult op1=add)
  on the vector engine, chained across 512-wide time chunks via `initial`.
- h lives in SBUF as 8 feature-blocks x 4 chunk tiles of [128, 512] f32r and
  is updated in place layer after layer (the scan of chunk c only runs after
  every matmul of chunk c has consumed the old h, which Tile enforces via
  WAR dependencies).
- The time axis is split in two groups of 2048 so that h (8.4MB) plus one
  layer of weights (8.4MB) fit in SBUF; per-layer carries [128,1] bridge the
  groups. Layer weights are streamed from DRAM into 16 single-buffered
  column tiles, which pipelines the next layer's loads behind the current
  layer's last reads.
"""

import numpy as np

import concourse.bass as bass
import concourse.mybir as mybir
import concourse.tile as tile

# ---------------------------------------------------------------------------
# Workaround: this walrus build accepts at most ONE sem wait per instruction
# ("Too many sync wait commands"). After Tile assigns waits, split any
# instruction carrying more by inserting same-engine NoOps ahead of it.
# ---------------------------------------------------------------------------
from concourse.vector_clock import ScopedClock

_MAX_WAITS = 1
_noop_ctr = [0]


def _split_waits_in_block(bb):
    new_list = []
    for inst in bb.instructions:
        si = getattr(inst, "sync_info", None)
        if si is not None and si.on_wait and len(si.on_wait) > _MAX_WAITS:
            waits = list(si.on_wait)
            keep = waits[-_MAX_WAITS:]
            extra = waits[:-_MAX_WAITS]
            for i in range(0, len(extra), _MAX_WAITS):
                _noop_ctr[0] += 1
                nop = mybir.InstNoOp(
                    name=f"I-waitsplit-{_noop_ctr[0]}",
                    engine=inst.engine,
                    bass_nofuse=True,
                    sync_info=mybir.SyncInfo(
                        on_wait=extra[i : i + _MAX_WAITS], on_update=[]
                    ),
                )
                new_list.append(nop)
            inst.sync_info = mybir.SyncInfo(on_wait=keep, on_update=si.on_update)
        new_list.append(inst)
    bb.instructions[:] = new_list


def _patched_drain_and_barrier(self, tick_clock, wait_clock):
    nc = self.nc
    drain_inst = nc.sync.drain()
    wait_clock.add_sem_waits(
        drain_inst.ins, ScopedClock({None: tick_clock.global_clock})
    )
    for bb in nc.main_func.blocks:
        _split_waits_in_block(bb)
    nc.all_engine_barrier()
    assert self.sems is not None
    popped = nc._tile_sem_poison_stack.pop()
    assert popped is self._sem_poison
    nc.clear_and_free_semaphores(list(self.sems.allocated().values()))
    nc.all_engine_barrier()


tile.TileContext._drain_and_barrier = _patched_drain_and_barrier

# ---------------------------------------------------------------------------

f32 = mybir.dt.float32
f32r = mybir.dt.float32r
AF = mybir.ActivationFunctionType
ALU = mybir.AluOpType

N_CORES = 8
B_FULL = 16
C_IN = 80
C_OUT = 194
D = 1024
NJ = D // 128  # 8 feature blocks of 128


def build_program(L=6, T=4096, G=2, S=512, BS=2, REP=1, mm_dt="f32r", fake_scan=False, static_w=False, layers_only=False):
    """Build the SPMD per-core Bass program. Returns nc.

    REP > 1 repeats the whole compute body (for differential timing); the
    program is idempotent so results are unchanged.
    """
    Tg = T // G
    NCH = Tg // S
    dmm = f32r if mm_dt == "f32r" else mybir.dt.bfloat16
    dio = f32 if mm_dt == "f32r" else mybir.dt.bfloat16
    nc = bass.Bass()

    x_d = nc.declare_dram_parameter("x", [BS, C_IN, T], dio, isOutput=False)
    wpre_d = nc.declare_dram_parameter("wpre", [C_IN, D], dio, isOutput=False)
    bpre_d = nc.declare_dram_parameter("bpre", [128, NJ], f32, isOutput=False)
    wl_d = nc.declare_dram_parameter("wl", [L, 2 * NJ, 128, D], dio, isOutput=False)
    bl_d = nc.declare_dram_parameter("bl", [L, 2, 128, NJ], f32, isOutput=False)
    wpost_d = nc.declare_dram_parameter(
        "wpost", [128, NJ * C_OUT], dio, isOutput=False
    )
    bpost_d = nc.declare_dram_parameter("bpost", [128, 2], f32, isOutput=False)
    out_d = nc.declare_dram_parameter("out", [BS, C_OUT, T], f32, isOutput=True)

    with tile.TileContext(nc) as tc:
        with (
            tc.tile_pool(name="const", bufs=1) as cpool,
            tc.tile_pool(name="h", bufs=1) as hpool,
            tc.tile_pool(name="w", bufs=1) as wpool,
            tc.tile_pool(name="bias", bufs=1) as bpool,
            tc.tile_pool(name="scr", bufs=1) as spool,
            tc.tile_pool(name="ps", bufs=1, space="PSUM") as pspool,
        ):
            # ---- constants loaded once ----
            wpre_sb = cpool.tile([C_IN, D], dmm, tag="wpre")
            nc.sync.dma_start(wpre_sb[:], wpre_d[:].bitcast(dmm))
            bpre_sb = cpool.tile([128, NJ], f32, tag="bpre")
            nc.sync.dma_start(bpre_sb[:], bpre_d[:])
            wpost_sb = cpool.tile([128, NJ * C_OUT], dmm, tag="wpost")
            nc.sync.dma_start(wpost_sb[:], wpost_d[:].bitcast(dmm))
            bpost_sb = cpool.tile([128, 2], f32, tag="bpost")
            nc.sync.dma_start(bpost_sb[:], bpost_d[:])
            carry_sb = cpool.tile([128, L * NJ], dmm, tag="carry")

            # persistent h tiles: [feature-block j][chunk c] of [128, S]
            h = [
                [hpool.tile([128, S], dmm, tag=f"h{j}_{c}", name=f"h{j}_{c}")
                 for c in range(NCH)]
                for j in range(NJ)
            ]

            for _rep in range(REP):
              for b in range(BS):
                for g in range(G):
                    t0 = g * Tg
                    # ---- input slab for this (batch, group) ----
                    x_sb = spool.tile([C_IN, Tg], dmm, tag="x", name="x_sb")
                    nc.sync.dma_start(
                        x_sb[:], x_d[b][:, t0 : t0 + Tg].bitcast(dmm)
                    )

                    # ---- pre-projection: h = x^T W_pre + b_pre (feature-major)
                    for c in range(NCH if not layers_only else 0):
                        for j in range(NJ):
                            ps = pspool.tile(
                                [128, S], f32,
                                tag=("psz" if j % 2 == 0 else "psc"), bufs=4,
                                name="ps_pre",
                            )
                            nc.tensor.matmul(
                                ps[:],
                                wpre_sb[:, j * 128 : (j + 1) * 128],
                                x_sb[:, c * S : (c + 1) * S],
                                start=True,
                                stop=True,
                            )
                            if j % 2 == 0:
                                nc.scalar.activation(
                                    h[j][c][:], ps[:], AF.Identity,
                                    bias=bpre_sb[:, j : j + 1], scale=1.0,
                                )
                            else:
                                nc.vector.tensor_scalar(
                                    h[j][c][:], ps[:],
                                    bpre_sb[:, j : j + 1], None, op0=ALU.add,
                                )

                    # ---- the L minGRU layers ----
                    for i in range(L):
                        if static_w and (b > 0 or g > 0 or i > 0 or _rep > 0):
                            pass  # reuse layer-0 weights (debug variant)
                        else:
                          wcols = []
                          for f in range(2 * NJ):
                            wt = wpool.tile(
                                [128, D], dmm, tag=f"w{f}", name=f"w{f}"
                            )
                            nc.sync.dma_start(wt[:], wl_d[i, f].bitcast(dmm))
                            wcols.append(wt)
                        bz = bpool.tile([128, NJ], f32, tag="bz", bufs=2, name="bz")
                        nc.sync.dma_start(bz[:], bl_d[i, 0])
                        bc = bpool.tile([128, NJ], f32, tag="bc", bufs=2, name="bc")
                        nc.sync.dma_start(bc[:], bl_d[i, 1])
                        nbz = bpool.tile([128, NJ], f32, tag="nbz", bufs=2, name="nbz")
                        nc.scalar.mul(nbz[:], bz[:], -1.0)

                        for c in range(NCH):
                            a_ts, b_ts = [], []
                            for j in range(NJ):
                                psz = pspool.tile(
                                    [128, S], f32, tag="psz", bufs=4, name="psz"
                                )
                                psc = pspool.tile(
                                    [128, S], f32, tag="psc", bufs=4, name="psc"
                                )
                                for kb in range(NJ):
                                    nc.tensor.matmul(
                                        psz[:],
                                        wcols[j][:, kb * 128 : (kb + 1) * 128],
                                        h[kb][c][:],
                                        start=(kb == 0),
                                        stop=(kb == NJ - 1),
                                    )
                                for kb in range(NJ):
                                    nc.tensor.matmul(
                                        psc[:],
                                        wcols[NJ + j][:, kb * 128 : (kb + 1) * 128],
                                        h[kb][c][:],
                                        start=(kb == 0),
                                        stop=(kb == NJ - 1),
                                    )
                                z_t = spool.tile(
                                    [128, S], f32, tag="z", bufs=4, name="z_t"
                                )
                                a_t = spool.tile(
                                    [128, S], f32, tag="a", bufs=9, name="a_t"
                                )
                                b_t = spool.tile(
                                    [128, S], f32, tag="bb", bufs=9, name="b_t"
                                )
                                # z = sigmoid(zh_z + bz)
                                nc.scalar.activation(
                                    z_t[:], psz[:], AF.Sigmoid,
                                    bias=bz[:, j : j + 1], scale=1.0,
                                )
                                # a = 1 - z = sigmoid(-(zh_z + bz))
                                nc.scalar.activation(
                                    a_t[:], psz[:], AF.Sigmoid,
                                    bias=nbz[:, j : j + 1], scale=-1.0,
                                )
                                # b = z * (zh_c + bc)
                                nc.vector.scalar_tensor_tensor(
                                    b_t[:], psc[:], bc[:, j : j + 1], z_t[:],
                                    op0=ALU.add, op1=ALU.mult,
                                )
                                a_ts.append(a_t)
                                b_ts.append(b_t)
                            # scans run after ALL matmuls of this chunk
                            for j in range(NJ):
                                if g == 0 and c == 0:
                                    init = 0.0
                                elif c == 0:
                                    init = carry_sb[:, i * NJ + j : i * NJ + j + 1]
                                else:
                                    init = h[j][c - 1][:, S - 1 : S]
                                if fake_scan:
                                    nc.vector.scalar_tensor_tensor(
                                        h[j][c][:], a_ts[j][:], 1.0, b_ts[j][:],
                                        op0=ALU.mult, op1=ALU.add,
                                    )
                                else:
                                    nc.vector.tensor_tensor_scan(
                                        h[j][c][:], a_ts[j][:], b_ts[j][:], init,
                                        op0=ALU.mult, op1=ALU.add,
                                    )
                        if g == 0:
                            for j in range(NJ):
                                nc.vector.tensor_copy(
                                    carry_sb[:, i * NJ + j : i * NJ + j + 1],
                                    h[j][NCH - 1][:, S - 1 : S],
                                )

                    # ---- post-projection: out = h^T W_post + b_post ----
                    if layers_only:
                        for c in range(NCH):
                            nc.sync.dma_start(
                                out_d[b][0:128, t0 + c * S : t0 + (c + 1) * S].bitcast(dmm),
                                h[0][c][:],
                            )
                        continue
                    for c in range(NCH):
                        for p, (p0, pw) in enumerate(((0, 128), (128, C_OUT - 128))):
                            ps_o = pspool.tile(
                                [128, S], f32,
                                tag=("psz" if p == 0 else "psc"), bufs=4,
                                name="ps_o",
                            )
                            for kb in range(NJ):
                                nc.tensor.matmul(
                                    ps_o[:pw, :],
                                    wpost_sb[
                                        :, kb * C_OUT + p0 : kb * C_OUT + p0 + pw
                                    ],
                                    h[kb][c][:],
                                    start=(kb == 0),
                                    stop=(kb == NJ - 1),
                                )
                            o_t = spool.tile([128, S], f32, tag="o", bufs=4, name="o_t")
                            if p == 0:
                                nc.scalar.activation(
                                    o_t[:pw, :], ps_o[:pw, :], AF.Identity,
                                    bias=bpost_sb[:pw, p : p + 1], scale=1.0,
                                )
                            else:
                                nc.vector.tensor_scalar(
                                    o_t[:pw, :], ps_o[:pw, :],
                                    bpost_sb[:pw, p : p + 1], None, op0=ALU.add,
                                )
                            nc.sync.dma_start(
                                out_d[b][p0 : p0 + pw, t0 + c * S : t0 + (c + 1) * S],
                                o_t[:pw, :],
                            )
    return nc


# ---------------------------------------------------------------------------
# fp8 DoubleRow variant
#
# Layer matmuls run in fp8e4m3 with the DoubleRow perf mode (2 fp8 k-elements
# per PE row -> 0.5 cycles/row, 2x the f32r rate).  Numerics (validated
# offline against the fp32 reference, max-rel ~3.5e-3 vs the 2e-2 gate):
#   - gate path zh_z:   plain fp8 W x fp8 h_hi            (4 DoubleRows)
#   - candidate zh_c:   split3 = Whi*hhi + Wlo*hhi + Whi*hlo (12 DoubleRows)
# where Xhi = fp8(X*scale), Xlo = fp8(X*scale - Xhi) (residual at same scale).
# Per-layer power-of-2 scales keep fp8 operands in range ~[-160, 160].
#
# All elementwise work is spread so no engine exceeds the PE's 4096
# cycles/(j,c)-tile:  Act: z=Sigmoid, c=Identity(+bias,dequant-scale);
# Pool: a=1-z, hhi=tensor_copy(h');  DVE: b=z*c, scan, hlo=h'-hhi.
# h' tiles are bf16 and live in SCALED units (h * 2^sh_next) so the fp8
# casts are pure copies; biases/scales are folded into Act's scale port.
# ---------------------------------------------------------------------------

f8 = mybir.dt.float8e4
bf16 = mybir.dt.bfloat16
u8 = mybir.dt.uint8
u16 = mybir.dt.uint16
NQ = NJ // 2  # 4 k-pairs of 256
SW = 10  # weight scale 2^10  (max|w|*2^10 ~ 107 < 240)
SH = [7, 8, 9, 10, 11, 12]  # per-layer input-h scales (max|h|*2^sh ~ 96-138)
DR = mybir.MatmulPerfMode.DoubleRow


def build_program_fp8(L=6, T=4096, G=2, S=512, BS=2, REP=1, pool_a=True,
                      cand_bf16=True):
    Tg = T // G
    NCH = Tg // S
    f32r = mybir.dt.float32r
    nc = bass.Bass()

    x_d = nc.declare_dram_parameter("x", [BS, C_IN, T], f32, isOutput=False)
    wpre_d = nc.declare_dram_parameter("wpre", [C_IN + 1, D], f32, isOutput=False)
    wz_d = nc.declare_dram_parameter("wz8", [L, 128, NQ * 2 * D], u8, isOutput=False)
    if cand_bf16:
        wcb_d = nc.declare_dram_parameter(
            "wcb", [L, 128, NJ * D], u16, isOutput=False
        )
    else:
        wch_d = nc.declare_dram_parameter(
            "wch8", [L, 128, NQ * 2 * D], u8, isOutput=False
        )
        wcl_d = nc.declare_dram_parameter(
            "wcl8", [L, 128, NQ * 2 * D], u8, isOutput=False
        )
    bz_d = nc.declare_dram_parameter("bz", [L, 128, NJ], f32, isOutput=False)
    bc_d = nc.declare_dram_parameter("bcs", [L, 128, NJ], f32, isOutput=False)
    wpost_d = nc.declare_dram_parameter("wpost", [128, NJ * C_OUT], u16, isOutput=False)
    bpost_d = nc.declare_dram_parameter("bpost", [1, C_OUT], u16, isOutput=False)
    out_d = nc.declare_dram_parameter("out", [BS, C_OUT, T], f32, isOutput=True)

    with tile.TileContext(nc) as tc:
        with (
            tc.tile_pool(name="const", bufs=1) as cpool,
            tc.tile_pool(name="h", bufs=1) as hpool,
            tc.tile_pool(name="h8", bufs=1) as h8pool,
            tc.tile_pool(name="w", bufs=2) as wpool,
            tc.tile_pool(name="bias", bufs=2) as bpool,
            tc.tile_pool(name="scr", bufs=1) as spool,
            tc.tile_pool(name="ps", bufs=1, space="PSUM") as pspool,
        ):
            # ---- constants ----
            wpre_sb = cpool.tile([C_IN + 1, D], f32r, tag="wpre")
            nc.sync.dma_start(wpre_sb[:], wpre_d[:].bitcast(f32r))
            wpost_sb = cpool.tile([128, NJ * C_OUT], bf16, tag="wpost")
            nc.sync.dma_start(wpost_sb[:], wpost_d[:].bitcast(bf16))
            bpost_sb = cpool.tile([1, C_OUT], bf16, tag="bpost")
            nc.sync.dma_start(bpost_sb[:], bpost_d[:].bitcast(bf16))
            ones_sb = cpool.tile([1, S], bf16, tag="ones")
            nc.vector.memset(ones_sb[:], 1.0)
            carry_sb = cpool.tile([128, L * NJ], bf16, tag="carry")

            # persistent h' (bf16, scaled units) and fp8 hi tiles, double-
            # buffered by layer parity so next-layer-input writes never WAR
            # against this layer's matmul reads.  In cand_bf16 mode the
            # candidate matmuls read the bf16 h' tiles directly (so h' needs
            # real parity and no hlo exists); otherwise h' has no cross-layer
            # readers and both parities alias the same tiles.
            def h_tiles(p):
                return [
                    [hpool.tile([128, S], bf16, tag=f"h{p}_{j}_{c}",
                                name=f"h{p}_{j}_{c}") for c in range(NCH)]
                    for j in range(NJ)
                ]

            if cand_bf16:
                h = [h_tiles(0), h_tiles(1)]
            else:
                h0 = h_tiles(0)
                h = [h0, h0]
            hhi = [
                [h8pool.tile([128, NJ, S], f8, tag=f"hhi{p}_{c}",
                             name=f"hhi{p}_{c}") for c in range(NCH)]
                for p in range(2)
            ]
            if not cand_bf16:
                hlo = [
                    [h8pool.tile([128, NJ, S], f8, tag=f"hlo{p}_{c}",
                                 name=f"hlo{p}_{c}") for c in range(NCH)]
                    for p in range(2)
                ]

            slab = 0
            for _rep in range(REP):
              for b in range(BS):
                for g in range(G):
                    t0 = g * Tg
                    x_sb = spool.tile([C_IN + 1, Tg], f32r, tag="x", bufs=2,
                                      name="x_sb")
                    if slab < 2:  # ones row persists per rotating buffer
                        # (memset must start at partition 0; row C_IN alone
                        # is not a legal base partition; f32r memset is not
                        # a valid ISA value type, so memset the f32 view)
                        nc.vector.memset(x_sb[:, :].bitcast(f32), 1.0)
                    nc.sync.dma_start(
                        x_sb[:C_IN, :], x_d[b][:, t0 : t0 + Tg].bitcast(f32r)
                    )
                    slab += 1

                    # ---- pre-projection (f32r): h0' = (x^T Wpre + bpre)*2^sh0
                    for c in range(NCH):
                        for j in range(NJ):
                            ps = pspool.tile(
                                [128, S], f32,
                                tag=(f"psz{c}" if j % 2 == 0 else f"psc{c}"),
                                bufs=1, name="ps_pre",
                            )
                            nc.tensor.matmul(
                                ps[:],
                                wpre_sb[:, j * 128 : (j + 1) * 128],
                                x_sb[:, c * S : (c + 1) * S],
                                start=True,
                                stop=True,
                            )
                            nc.scalar.activation(
                                h[0][j][c][:], ps[:], AF.Copy,
                                scale=float(2.0 ** SH[0]),
                            )
                            nc.gpsimd.tensor_copy(
                                hhi[0][c][:, j, :], h[0][j][c][:]
                            )
                            if not cand_bf16:
                                nc.vector.tensor_tensor(
                                    hlo[0][c][:, j, :], h[0][j][c][:],
                                    hhi[0][c][:, j, :], op=ALU.subtract,
                                )

                    # ---- the L minGRU layers ----
                    for i in range(L):
                        wz_sb = wpool.tile([128, NQ, 2, D], f8, tag="wz", name="wz")
                        nc.sync.dma_start(wz_sb[:], wz_d[i].bitcast(f8))
                        if cand_bf16:
                            wcb_sb = wpool.tile([128, NJ * D], bf16, tag="wcb",
                                                name="wcb")
                            nc.sync.dma_start(wcb_sb[:], wcb_d[i].bitcast(bf16))
                        else:
                            wch_sb = wpool.tile([128, NQ, 2, D], f8, tag="wch",
                                                name="wch")
                            nc.sync.dma_start(wch_sb[:], wch_d[i].bitcast(f8))
                            wcl_sb = wpool.tile([128, NQ, 2, D], f8, tag="wcl",
                                                name="wcl")
                            nc.sync.dma_start(wcl_sb[:], wcl_d[i].bitcast(f8))
                        bz_sb = bpool.tile([128, NJ], f32, tag="bz", name="bz")
                        nc.sync.dma_start(bz_sb[:], bz_d[i])
                        bc_sb = bpool.tile([128, NJ], f32, tag="bc", name="bc")
                        nc.sync.dma_start(bc_sb[:], bc_d[i])

                        sc_in = float(2.0 ** (-(SW + SH[i])))
                        sh_next = SH[i + 1] if i + 1 < L else 0
                        if cand_bf16:
                            sc_c = float(2.0 ** (sh_next - SH[i]))
                        else:
                            sc_c = float(2.0 ** (sh_next - SW - SH[i]))
                        last = i == L - 1
                        rd, wr = i % 2, (i + 1) % 2

                        for j in range(NJ):
                            jsl = slice(j * 128, (j + 1) * 128)
                            psz_t = [
                                pspool.tile([128, S], f32, tag=f"psz{c}",
                                            bufs=1, name=f"psz{c}")
                                for c in range(NCH)
                            ]
                            psc_t = [
                                pspool.tile([128, S], f32, tag=f"psc{c}",
                                            bufs=1, name=f"psc{c}")
                                for c in range(NCH)
                            ]

                            def pair(t, c, q):
                                return t[rd][c][:, 2 * q : 2 * q + 2, :]

                            # gate: Wz x hhi -- each weight slice loaded once,
                            # reused across the NCH chunks (ldweights=False)
                            for q in range(NQ):
                                for c in range(NCH):
                                    mm = nc.tensor.matmul(
                                        psz_t[c][:], wz_sb[:, q, :, jsl],
                                        pair(hhi, c, q),
                                        start=(q == 0), stop=(q == NQ - 1),
                                        perf_mode=DR,
                                    )
                                    if c > 0:
                                        mm.ins.ldweights = False
                            if cand_bf16:
                                # candidate: bf16 W x bf16 h' (read directly)
                                for kb in range(NJ):
                                    wsl = wcb_sb[
                                        :, kb * D + j * 128 : kb * D + (j + 1) * 128
                                    ]
                                    for c in range(NCH):
                                        mm = nc.tensor.matmul(
                                            psc_t[c][:], wsl, h[rd][kb][c][:],
                                            start=(kb == 0),
                                            stop=(kb == NJ - 1),
                                        )
                                        if c > 0:
                                            mm.ins.ldweights = False
                            else:
                                # candidate phase 1: Wc_hi x (hhi then hlo) --
                                # same weight load covers both products
                                for q in range(NQ):
                                    for k, hsrc in enumerate((hhi, hlo)):
                                        for c in range(NCH):
                                            mm = nc.tensor.matmul(
                                                psc_t[c][:], wch_sb[:, q, :, jsl],
                                                pair(hsrc, c, q),
                                                start=(q == 0 and k == 0),
                                                stop=False,
                                                perf_mode=DR,
                                            )
                                            if k > 0 or c > 0:
                                                mm.ins.ldweights = False
                                # candidate phase 2: Wc_lo x hhi
                                for q in range(NQ):
                                    for c in range(NCH):
                                        mm = nc.tensor.matmul(
                                            psc_t[c][:], wcl_sb[:, q, :, jsl],
                                            pair(hhi, c, q),
                                            start=False, stop=(q == NQ - 1),
                                            perf_mode=DR,
                                        )
                                        if c > 0:
                                            mm.ins.ldweights = False

                            for c in range(NCH):
                                z_t = spool.tile(
                                    [128, S], bf16, tag="z", bufs=4, name="z_t"
                                )
                                c_t = spool.tile(
                                    [128, S], bf16, tag="cc", bufs=4, name="c_t"
                                )
                                a_t = spool.tile(
                                    [128, S], bf16, tag="a", bufs=5, name="a_t"
                                )
                                b_t = spool.tile(
                                    [128, S], bf16, tag="bb", bufs=5, name="b_t"
                                )
                                nc.scalar.activation(
                                    z_t[:], psz_t[c][:], AF.Sigmoid,
                                    bias=bz_sb[:, j : j + 1], scale=sc_in,
                                )
                                nc.scalar.activation(
                                    c_t[:], psc_t[c][:], AF.Identity,
                                    bias=bc_sb[:, j : j + 1], scale=sc_c,
                                )
                                if pool_a:
                                    nc.gpsimd.tensor_scalar(
                                        a_t[:], z_t[:], -1.0, 1.0,
                                        op0=ALU.mult, op1=ALU.add,
                                    )
                                else:
                                    nc.vector.tensor_scalar(
                                        a_t[:], z_t[:], -1.0, 1.0,
                                        op0=ALU.mult, op1=ALU.add,
                                    )
                                nc.vector.tensor_tensor(
                                    b_t[:], z_t[:], c_t[:], op=ALU.mult
                                )
                                if g == 0 and c == 0:
                                    init = 0.0
                                elif c == 0:
                                    init = carry_sb[:, i * NJ + j : i * NJ + j + 1]
                                else:
                                    init = h[wr][j][c - 1][:, S - 1 : S]
                                nc.vector.tensor_tensor_scan(
                                    h[wr][j][c][:], a_t[:], b_t[:], init,
                                    op0=ALU.mult, op1=ALU.add,
                                )
                                if not last:
                                    nc.gpsimd.tensor_copy(
                                        hhi[wr][c][:, j, :], h[wr][j][c][:]
                                    )
                                    if not cand_bf16:
                                        nc.vector.tensor_tensor(
                                            hlo[wr][c][:, j, :], h[wr][j][c][:],
                                            hhi[wr][c][:, j, :], op=ALU.subtract,
                                        )
                            if g == 0:
                                nc.vector.tensor_copy(
                                    carry_sb[:, i * NJ + j : i * NJ + j + 1],
                                    h[wr][j][NCH - 1][:, S - 1 : S],
                                )

                    # ---- post-projection (bf16): out = h^T Wpost + bpost ----
                    for c in range(NCH):
                        for p, (p0, pw) in enumerate(((0, 128), (128, C_OUT - 128))):
                            ps_o = pspool.tile(
                                [128, S], f32,
                                tag=(f"psz{c}" if p == 0 else f"psc{c}"),
                                bufs=1, name="ps_o",
                            )
                            for kb in range(NJ):
                                nc.tensor.matmul(
                                    ps_o[:pw, :],
                                    wpost_sb[
                                        :, kb * C_OUT + p0 : kb * C_OUT + p0 + pw
                                    ],
                                    h[L % 2][kb][c][:],
                                    start=(kb == 0),
                                    stop=False,
                                )
                            nc.tensor.matmul(
                                ps_o[:pw, :],
                                bpost_sb[:, p0 : p0 + pw],
                                ones_sb[:],
                                start=False,
                                stop=True,
                            )
                            o_t = spool.tile([128, S], f32, tag="o", bufs=4,
                                             name="o_t")
                            nc.scalar.activation(
                                o_t[:pw, :], ps_o[:pw, :], AF.Copy, scale=1.0
                            )
                            nc.sync.dma_start(
                                out_d[b][p0 : p0 + pw, t0 + c * S : t0 + (c + 1) * S],
                                o_t[:pw, :],
                            )
    return nc


def pack_inputs_fp8(x, w_pre, b_pre, w_layers, b_layers, w_post, b_post, L=6,
                    cand_bf16=True):
    import ml_dtypes

    fp8np = ml_dtypes.float8_e4m3
    bf16np = ml_dtypes.bfloat16
    f32 = np.float32

    x = np.ascontiguousarray(np.asarray(x, dtype=f32))
    wpre81 = np.empty((C_IN + 1, D), dtype=f32)
    wpre81[:C_IN] = np.asarray(w_pre, dtype=f32)
    wpre81[C_IN] = np.asarray(b_pre, dtype=f32)

    wl = np.asarray(w_layers, dtype=f32)
    sw = 2.0 ** SW

    def pack_w(wm):  # wm: [L, D, D] -> [L, 128, NQ*2*D] uint8 view of fp8
        # fp8 values already computed; layout [p, q, pair, m]
        out = np.empty((L, 128, NQ * 2 * D), dtype=np.uint8)
        for i in range(L):
            wi = wm[i]  # [D, D] fp8
            r = wi.reshape(NQ, 2, 128, D).transpose(2, 0, 1, 3).reshape(128, -1)
            out[i] = np.ascontiguousarray(r).view(np.uint8)
        return out

    wz_f = wl[:, :, :D] * sw
    wz8 = pack_w(wz_f.astype(fp8np))
    if cand_bf16:
        # [L, 128, NJ*D] bf16: wcb[i][p, kb*D+m] = wc[i][kb*128+p, m]
        wcb = (
            wl[:, :, D:]
            .reshape(L, NJ, 128, D)
            .transpose(0, 2, 1, 3)
            .reshape(L, 128, NJ * D)
        )
        wcb = np.ascontiguousarray(wcb.astype(bf16np)).view(np.uint16)
    else:
        wc_f = wl[:, :, D:] * sw
        wch8 = wc_f.astype(fp8np)
        wcl8 = (wc_f - wch8.astype(f32)).astype(fp8np)
        wch8p = pack_w(wch8)
        wcl8p = pack_w(wcl8)

    bl = np.asarray(b_layers, dtype=f32)  # [L, 2D]
    bz = np.ascontiguousarray(
        bl[:, :D].reshape(L, NJ, 128).transpose(0, 2, 1)
    )  # [L, 128, NJ]
    bcs = bl[:, D:].reshape(L, NJ, 128).transpose(0, 2, 1).copy()
    for i in range(L):
        sh_next = SH[i + 1] if i + 1 < L else 0
        bcs[i] *= 2.0 ** sh_next
    bcs = np.ascontiguousarray(bcs)

    wpost = (
        np.asarray(w_post, dtype=f32)
        .reshape(NJ, 128, C_OUT)
        .transpose(1, 0, 2)
        .reshape(128, NJ * C_OUT)
    )
    wpost = np.ascontiguousarray(wpost.astype(bf16np)).view(np.uint16)
    bpost = np.ascontiguousarray(
        np.asarray(b_post, dtype=f32).reshape(1, C_OUT).astype(bf16np)
    ).view(np.uint16)
    out = {
        "x": x, "wpre": wpre81, "wz8": wz8,
        "bz": bz, "bcs": bcs, "wpost": wpost, "bpost": bpost,
    }
    if cand_bf16:
        out["wcb"] = wcb
    else:
        out["wch8"] = wch8p
        out["wcl8"] = wcl8p
    return out


def pack_inputs(x, w_pre, b_pre, w_layers, b_layers, w_post, b_post, L=6, mm_dt="f32r"):
    """Host-side packing into DMA-friendly layouts (all contiguous 2D)."""
    if mm_dt == "f32r":
        io_np = np.float32
    else:
        import ml_dtypes
        io_np = ml_dtypes.bfloat16
    x = np.ascontiguousarray(np.asarray(x, dtype=np.float32).astype(io_np))
    w_pre = np.ascontiguousarray(np.asarray(w_pre, dtype=np.float32).astype(io_np))
    # wl[i, f, kp, kb*128+m] = w_layers[i, kb*128+kp, f*128+m]
    wl = (
        np.asarray(w_layers, dtype=np.float32)
        .reshape(L, NJ, 128, 2 * NJ, 128)
        .transpose(0, 3, 2, 1, 4)
        .reshape(L, 2 * NJ, 128, D)
    )
    wl = np.ascontiguousarray(wl.astype(io_np))
    bl = np.asarray(b_layers, dtype=np.float32).reshape(L, 2, NJ, 128)
    bl = np.ascontiguousarray(bl.transpose(0, 1, 3, 2))  # [L, 2, 128, NJ]
    bpre = np.ascontiguousarray(
        np.asarray(b_pre, dtype=np.float32).reshape(NJ, 128).T
    )  # [128, NJ]
    # wpost[kp, kb*C_OUT + c] = w_post[kb*128+kp, c]
    wpost = (
        np.asarray(w_post, dtype=np.float32)
        .reshape(NJ, 128, C_OUT)
        .transpose(1, 0, 2)
        .reshape(128, NJ * C_OUT)
    )
    wpost = np.ascontiguousarray(wpost.astype(io_np))
    bpost = np.zeros((128, 2), dtype=np.float32)
    bpost[:, 0] = np.asarray(b_post[:128], dtype=np.float32)
    bpost[: C_OUT - 128, 1] = np.asarray(b_post[128:], dtype=np.float32)
    return x, w_pre, bpre, wl, bl, wpost, bpost


_program_cache = {}


def _get_program(key):
    if key not in _program_cache:
        L, T, G, S, BS, REP, mm_dt = key[:7]
        fake = key[7] if len(key) > 7 else False
        statw = key[8] if len(key) > 8 else False
        lonly = key[9] if len(key) > 9 else False
        _program_cache[key] = build_program(L=L, T=T, G=G, S=S, BS=BS, REP=REP, mm_dt=mm_dt, fake_scan=fake, static_w=statw, layers_only=lonly)
    return _program_cache[key]


def run(inputs, L=6, T=4096, G=2, S=512, REP=1, mm_dt="fp8", trace=False,
        pool_a=True, **kw):
    """Run the SPMD kernel on the full inputs; returns (out, bass_results)."""
    from concourse.bass_utils import run_bass_kernel_spmd

    if mm_dt == "fp8":
        cand_bf16 = kw.pop("cand_bf16", True)
        packed = pack_inputs_fp8(
            inputs["x"], inputs["w_pre"], inputs["b_pre"], inputs["w_layers"],
            inputs["b_layers"], inputs["w_post"], inputs["b_post"], L=L,
            cand_bf16=cand_bf16,
        )
        x = packed.pop("x")
        B = x.shape[0]
        BS = B // N_CORES
        key = (L, T, G, S, BS, REP, "fp8", pool_a, cand_bf16)
        if key not in _program_cache:
            _program_cache[key] = build_program_fp8(
                L=L, T=T, G=G, S=S, BS=BS, REP=REP, pool_a=pool_a,
                cand_bf16=cand_bf16,
            )
        nc = _program_cache[key]
        in_maps = [
            {"x": np.ascontiguousarray(x[c * BS : (c + 1) * BS]), **packed}
            for c in range(N_CORES)
        ]
    else:
        x, w_pre, bpre, wl, bl, wpost, bpost = pack_inputs(
            inputs["x"], inputs["w_pre"], inputs["b_pre"], inputs["w_layers"],
            inputs["b_layers"], inputs["w_post"], inputs["b_post"], L=L,
            mm_dt=mm_dt,
        )
        B = x.shape[0]
        BS = B // N_CORES
        nc = _get_program((L, T, G, S, BS, REP, mm_dt))
        shared = {"wpre": w_pre, "bpre": bpre, "wl": wl, "bl": bl,
                  "wpost": wpost, "bpost": bpost}
        in_maps = [
            {"x": np.ascontiguousarray(x[c * BS : (c + 1) * BS]), **shared}
            for c in range(N_CORES)
        ]
    res = run_bass_kernel_spmd(
        nc, in_maps, list(range(N_CORES)), trace=trace, **kw
    )
    out = np.concatenate([res.results[c]["out"] for c in range(N_CORES)], axis=0)
    return out, res


def kernel(**inputs):
    out, _ = run(inputs)
    return out

